# revision 2
# baseline (speedup 1.0000x reference)
"""Trainium2 Bass kernel: CausalSelfAttention (B=1, T=2048, C=4096, H=32, HS=128, NE=32).

Tensor-parallel over heads: 4 heads/core on 8 cores. Single streamed x pass
with all QKV weights resident. Attention S-tiles are *woven* with QKV (and
later projection) matmuls so the exp on the Activation engine never throttles
the PE. Masking is a post-exp multiplicative triangular mask (off the
S->exp critical chain). Softmax denominator via 1-column matmuls + tiny PE
transposes + PE broadcast. Warmup matmuls ramp the PE p-state during the
initial weight DMA.
"""

import sys

sys.path.insert(0, "/opt/trn_rl_repo")

from collections import deque

import numpy as np
import ml_dtypes

import concourse.bass as bass
import concourse.bacc as bacc
import concourse.mybir as mybir
from concourse import tile
from concourse.bass_utils import run_bass_kernel_spmd

BF16 = mybir.dt.bfloat16
F32 = mybir.dt.float32

B, T, C = 1, 2048, 4096
H, HS, NE = 32, 128, 32
NCORES = 8
HL = H // NCORES
SCALE = 1.0 / float(np.sqrt(HS))

NQC = 4
ROT32 = list(range(16, 32)) + list(range(16))
XCHUNKS = [(0, 512)] + [(512 + 256 * i, 256) for i in range(6)]


def _build_program(repeat=1, collective=True):
    nc = bacc.Bacc(
        "TRN2",
        target_bir_lowering=False,
        debug=False,
        num_devices=NCORES if collective else 1,
    )

    xT = nc.dram_tensor("xT", [C, T], BF16, kind="ExternalInput")
    w_qkT = nc.dram_tensor("w_qkT", [C, 2 * HL * 128], BF16, kind="ExternalInput")
    w_vT = nc.dram_tensor("w_vT", [C, HL * 128], BF16, kind="ExternalInput")
    w_pT = nc.dram_tensor("w_pT", [C, 512], BF16, kind="ExternalInput")
    b_qk = nc.dram_tensor("b_qk", [128, 2 * HL], F32, kind="ExternalInput")
    b_v = nc.dram_tensor("b_v", [128, HL * 128], BF16, kind="ExternalInput")
    b_p = nc.dram_tensor("b_p", [128, 4], F32, kind="ExternalInput")
    cosT = nc.dram_tensor("cosT", [NE, T], BF16, kind="ExternalInput")
    sin_pm = nc.dram_tensor("sin_pm", [NE, T], BF16, kind="ExternalInput")
    tri128 = nc.dram_tensor("tri128", [128, 128], BF16, kind="ExternalInput")
    ident128 = nc.dram_tensor("ident128", [128, 128], BF16, kind="ExternalInput")
    outT = nc.dram_tensor("outT", [512, T], F32, kind="ExternalOutput")

    y_ins = [
        [nc.dram_tensor(f"y_in_{lh}_{qc}", [128, 512], BF16) for qc in range(NQC)]
        for lh in range(HL)
    ]
    y_outs = [
        [
            nc.dram_tensor(
                f"y_out_{lh}_{qc}", [NCORES * 128, 512], BF16, addr_space="Shared"
            )
            for qc in range(NQC)
        ]
        for lh in range(HL)
    ]

    xT_r = xT.ap().rearrange("(ct p) t -> p ct t", p=128)
    wqk_r = w_qkT.ap().rearrange("(ct p) r -> p ct r", p=128)
    wv_r = w_vT.ap().rearrange("(ct p) r -> p ct r", p=128)
    wp_r = w_pT.ap().rearrange("(ct p) r -> p ct r", p=128)
    yout_rs = [
        [y.ap().rearrange("(c p) t -> p c t", p=128) for y in row] for row in y_outs
    ]

    with tile.TileContext(nc) as tc:
      for _rep in range(repeat):
        with (
            tc.tile_pool(name="persist", bufs=1) as P0,
            tc.tile_pool(name="qtp", bufs=2) as QTP,
            tc.tile_pool(name="ytile", bufs=5) as YP,
            tc.tile_pool(name="stat", bufs=1) as ST,
            tc.tile_pool(name="pp1", bufs=24) as PP1,
            tc.tile_pool(name="psQK", bufs=1, space="PSUM") as psQK,
            tc.tile_pool(name="psS", bufs=2, space="PSUM") as psS,
            tc.tile_pool(name="psBig", bufs=2, space="PSUM") as psBig,
            tc.tile_pool(name="psDT", bufs=1, space="PSUM") as psDT,
            tc.tile_pool(name="psRow", bufs=1, space="PSUM") as psRow,
            tc.tile_pool(name="psB", bufs=1, space="PSUM") as psB,
        ):
            kT_sb = P0.tile([128, HL, T], BF16, tag="kT")
            v_sb = P0.tile([128, T // 128, 512], BF16, tag="v")
            bp_sb = P0.tile([128, 4], F32, tag="bp")
            tri_sb = P0.tile([128, 128], BF16, tag="tri")
            id_sb = P0.tile([128, 128], BF16, tag="id")
            ones_c = P0.tile([128, 1], BF16, tag="onc")
            ones_r = P0.tile([1, 128], BF16, tag="onr")
            warm_sb = P0.tile([128, 256], BF16, tag="warm")

            qT = {}

            # ---- weave machinery: filler generators yield after PE quanta ----
            filler = deque()

            def pull(n=1):
                for _ in range(n):
                    while filler:
                        try:
                            next(filler[0])
                            return
                        except StopIteration:
                            filler.popleft()
                    return

            def drain_filler():
                while filler:
                    try:
                        next(filler[0])
                    except StopIteration:
                        filler.popleft()

            def attn_chunk(qc, PP, pull_n, deferred=None):
                nkt = 4 * qc + 4
                p_tiles = {}

                def S_head(lh, kts):
                    for kt in kts:
                        r = kt - 4 * qc
                        c0 = 128 * r if r >= 1 else 0
                        ps = psS.tile([128, 512], F32, tag="S")
                        nc.tensor.matmul(
                            ps[:, c0:512],
                            lhsT=kT_sb[:, lh, kt * 128 : kt * 128 + 128],
                            rhs=qT[qc][:, lh, c0:512],
                            start=True, stop=True,
                        )
                        pt = PP.tile([128, 512], BF16, tag="P")
                        nc.scalar.activation(
                            pt[:, c0:512], ps[:, c0:512],
                            mybir.ActivationFunctionType.Exp,
                            scale=SCALE,
                        )
                        if r >= 0:
                            # multiplicative causal mask, off the exp chain
                            nc.vector.tensor_tensor(
                                pt[:, 128 * r : 128 * r + 128],
                                pt[:, 128 * r : 128 * r + 128],
                                tri_sb, op=mybir.AluOpType.mult,
                            )
                            if qc == 0 and r >= 1:
                                nc.gpsimd.memset(pt[:, 0 : 128 * r], 0.0)
                        p_tiles[(lh, kt)] = (pt, c0)
                        pull(pull_n)

                def DY_head(lh):
                    # column-major: one sequential accumulation group per
                    # psdt column (interleaved groups in one bank are not
                    # supported by the hardware accumulation tracker)
                    psdt = psDT.tile([128, 4], F32, tag="DT")
                    for s in range(4):
                        kts = [
                            kt for kt in range(nkt)
                            if p_tiles[(lh, kt)][1] // 128 <= s
                        ]
                        for i, kt in enumerate(kts):
                            pt, _ = p_tiles[(lh, kt)]
                            nc.tensor.matmul(
                                psdt[:, s : s + 1],
                                lhsT=pt[:, 128 * s : 128 * s + 128],
                                rhs=ones_c,
                                start=(i == 0), stop=(i == len(kts) - 1),
                            )
                    rcp32 = ST.tile([128, 4], F32, tag="rcp32")
                    nc.vector.reciprocal(rcp32, psdt)
                    rcpb = ST.tile([128, 4], BF16, tag="rcpb")
                    nc.vector.tensor_copy(rcpb, rcp32)
                    if qc == 0:
                        y_order = [
                            (kt, 0, kt == 0, kt == nkt - 1) for kt in range(nkt)
                        ]
                    else:
                        y_order = [(4 * qc, 0, True, False)]
                        for r in range(1, 4):
                            y_order.append((4 * qc + r, 128 * r, False, False))
                        for kt in range(4 * qc):
                            y_order.append((kt, 0, False, kt == 4 * qc - 1))
                    ys = psBig.tile([128, 512], F32, tag="big")
                    for kt, c0, st_, sp_ in y_order:
                        pt, _ = p_tiles[(lh, kt)]
                        nc.tensor.matmul(
                            ys[:, c0:512],
                            lhsT=v_sb[:, kt, lh * 128 : lh * 128 + 128],
                            rhs=pt[:, c0:512],
                            start=st_, stop=sp_,
                        )
                    psrow = psRow.tile([1, 512], BF16, tag="row")
                    for s in range(4):
                        nc.tensor.transpose(
                            psrow[:, 128 * s : 128 * s + 128],
                            rcpb[:, s : s + 1],
                            id_sb,
                        )
                    row_sb = ST.tile([1, 512], BF16, tag="row_sb")
                    nc.vector.tensor_copy(row_sb, psrow)
                    psb = psB.tile([128, 512], F32, tag="B")
                    nc.tensor.matmul(
                        psb, lhsT=ones_r, rhs=row_sb, start=True, stop=True
                    )
                    bcs = ST.tile([128, 512], BF16, tag="bcs")
                    nc.scalar.copy(bcs, psb)
                    yt = YP.tile([128, 512], BF16, tag="yt")
                    nc.vector.tensor_tensor(yt, ys, bcs, op=mybir.AluOpType.mult)

                    def gather(lh=lh):
                        nc.sync.dma_start(y_ins[lh][qc].ap(), yt)
                        if collective:
                            nc.gpsimd.collective_compute(
                                "AllGather",
                                mybir.AluOpType.bypass,
                                replica_groups=[list(range(NCORES))],
                                ins=[y_ins[lh][qc].ap().opt()],
                                outs=[y_outs[lh][qc].ap().opt()],
                            )
                        else:
                            nc.sync.dma_start(
                                y_outs[lh][qc].ap()[0:128, :], y_ins[lh][qc].ap()
                            )

                    if deferred is None:
                        gather()
                    else:
                        deferred.append(gather)

                half = max(nkt // 2, 1)
                S_head(0, range(nkt))
                S_head(1, range(nkt))
                DY_head(0)
                S_head(2, range(nkt))
                DY_head(1)
                S_head(3, range(half))        # cap P-pool liveness
                DY_head(2)
                S_head(3, range(half, nkt))
                DY_head(3)
                drain_filler()

            # ============ block 1: QKV + attention chunks 0-2 ============
            with (
                tc.tile_pool(name="wpool", bufs=1) as WP,
                tc.tile_pool(name="xp", bufs=2) as XP,
                tc.tile_pool(name="rope", bufs=2) as RP,
            ):
                wqk_sb = WP.tile([128, 32, 1024], BF16, tag="wqk")
                wv_sb = WP.tile([128, 32, 512], BF16, tag="wv")
                bqk_sb = WP.tile([128, 2 * HL], F32, tag="bqk")
                bv_sb = WP.tile([128, HL * 128], BF16, tag="bv")

                def load_cs(t0):
                    cc = RP.tile([NE, 256], BF16, tag="ccos", name="cc_t")
                    ss = RP.tile([NE, 256], BF16, tag="csin", name="ss_t")
                    nc.sync.dma_start(cc, cosT.ap()[:, t0 : t0 + 256])
                    nc.sync.dma_start(ss, sin_pm.ap()[:, t0 : t0 + 256])
                    return cc, ss

                nc.vector.memset(warm_sb, 0.0)
                nc.vector.memset(ones_c, 1.0)
                nc.vector.memset(ones_r, 1.0)

                def rope_inplace(dest32, cc, ss):
                    n = dest32.shape[-1]
                    rot = RP.tile([NE, n], BF16, tag="rot")
                    nc.vector.stream_shuffle(rot, dest32, mask=ROT32)
                    tcos = RP.tile([NE, n], BF16, tag="tcos")
                    nc.vector.tensor_tensor(
                        tcos, dest32, cc, op=mybir.AluOpType.mult
                    )
                    tsin = RP.tile([NE, n], BF16, tag="tsin")
                    nc.vector.tensor_tensor(
                        tsin, rot, ss, op=mybir.AluOpType.mult
                    )
                    nc.vector.tensor_tensor(
                        dest32, tcos, tsin, op=mybir.AluOpType.add
                    )

                def qk_drain(ps, rt, ts, t0, tn, cs_tiles):
                    lh, is_k = rt // 2, rt % 2
                    if is_k:
                        dsl = kT_sb[:, lh, ts]
                    else:
                        q0 = t0 % 512
                        dsl = qT[t0 // 512][:, lh, q0 : q0 + tn]
                    nc.scalar.activation(
                        dsl, ps,
                        mybir.ActivationFunctionType.Identity,
                        bias=bqk_sb[:, rt : rt + 1],
                    )
                    for hi, h0 in enumerate(range(0, tn, 256)):
                        cc, ss = cs_tiles[hi]
                        rope_inplace(dsl[0:NE, h0 : h0 + 256], cc, ss)

                # ---- chunk 0 (512 tokens, paced against the weight DMA) ----
                def run_chunk0():
                    wps = psB.tile([128, 512], F32, tag="B", name="warmps")
                    for _ in range(26):
                        nc.tensor.matmul(
                            wps[:, 0:256], lhsT=warm_sb[:, 0:128], rhs=warm_sb,
                            start=True, stop=True,
                        )
                    xt0a = XP.tile([128, 32, 256], BF16, tag="xt", name="xt0a")
                    xt0b = XP.tile([128, 32, 256], BF16, tag="xt", name="xt0b")
                    for g in range(8):
                        gs = slice(4 * g, 4 * g + 4)
                        nc.sync.dma_start(wqk_sb[:, gs, :], wqk_r[:, gs, :])
                        nc.scalar.dma_start(xt0a[:, gs, :], xT_r[:, gs, 0:256])
                        nc.scalar.dma_start(xt0b[:, gs, :], xT_r[:, gs, 256:512])
                        if g == 0:
                            # constants needed by the first qk drains
                            nc.sync.dma_start(bqk_sb, b_qk.ap())
                            cs0 = [load_cs(0), load_cs(256)]
                    for g in range(4):
                        gs = slice(8 * g, 8 * g + 8)
                        nc.sync.dma_start(wv_sb[:, gs, :], wv_r[:, gs, :])
                        if g == 0:
                            nc.sync.dma_start(bv_sb, b_v.ap())
                    nc.sync.dma_start(tri_sb, tri128.ap())
                    nc.sync.dma_start(id_sb, ident128.ap())
                    nc.sync.dma_start(bp_sb, b_p.ap())

                    qT[0] = QTP.tile([128, HL, 512], BF16, tag="qT", name="qt_c")
                    for rts, pools in (
                        (range(0, 6),
                         ((psS, "S"), (psS, "S"), (psBig, "big"),
                          (psBig, "big"), (psB, "B"), (psQK, "qk"))),
                        (range(6, 8), ((psS, "S"), (psS, "S"))),
                    ):
                        tiles = {}
                        for rt, (pool, ptag) in zip(rts, pools):
                            tiles[rt] = pool.tile(
                                [128, 512], F32, tag=ptag, name=f"c0ps{rt}",
                            )
                        # one sequential accumulation group per column half
                        for hb, xs in ((0, xt0a), (1, xt0b)):
                            for g in range(8):
                                for rt in rts:
                                    for ct in range(4 * g, 4 * g + 4):
                                        nc.tensor.matmul(
                                            tiles[rt][:, 256 * hb : 256 * hb + 256],
                                            lhsT=wqk_sb[:, ct,
                                                        rt * 128 : rt * 128 + 128],
                                            rhs=xs[:, ct, :],
                                            start=(ct == 0), stop=(ct == 31),
                                        )
                        for rt in rts:
                            qk_drain(tiles[rt], rt, slice(0, 512), 0, 512, cs0)
                    for st in range(4):
                        ps = psBig.tile([128, 512], F32, tag="big", name=f"c0v{st}")
                        xs = (xt0a, xt0b)[st // 2]
                        x0 = (st % 2) * 128
                        for ct in range(32):
                            nc.tensor.matmul(
                                ps,
                                lhsT=xs[:, ct, x0 : x0 + 128],
                                rhs=wv_sb[:, ct, :],
                                start=(ct == 0), stop=(ct == 31),
                            )
                        nc.vector.tensor_tensor(
                            v_sb[:, st, :], ps, bv_sb, op=mybir.AluOpType.add
                        )

                run_chunk0()

                def gen_qkv(xc):
                    t0, tn = XCHUNKS[xc]
                    ts = slice(t0, t0 + tn)
                    if t0 % 512 == 0:
                        qT[t0 // 512] = QTP.tile(
                            [128, HL, 512], BF16, tag="qT", name="qt_c"
                        )
                    xt = XP.tile([128, 32, 256], BF16, tag="xt")
                    nc.scalar.dma_start(xt, xT_r[:, :, ts])
                    cs_t = [load_cs(t0)]
                    for rt in range(8):
                        ps = psQK.tile([128, 256], F32, tag="qk")
                        for ct in range(32):
                            nc.tensor.matmul(
                                ps,
                                lhsT=wqk_sb[:, ct, rt * 128 : rt * 128 + 128],
                                rhs=xt[:, ct, :],
                                start=(ct == 0), stop=(ct == 31),
                            )
                            if ct % 6 == 5:
                                yield
                        qk_drain(ps, rt, ts, t0, tn, cs_t)
                        yield
                    for sti in range(tn // 128):
                        st = t0 // 128 + sti
                        ps = psBig.tile([128, 512], F32, tag="big")
                        for ct in range(32):
                            nc.tensor.matmul(
                                ps,
                                lhsT=xt[:, ct, sti * 128 : sti * 128 + 128],
                                rhs=wv_sb[:, ct, :],
                                start=(ct == 0), stop=(ct == 31),
                            )
                            if ct % 3 == 2:
                                yield
                        nc.vector.tensor_tensor(
                            v_sb[:, st, :], ps, bv_sb, op=mybir.AluOpType.add
                        )
                        yield

                filler.append(gen_qkv(1))
                filler.append(gen_qkv(2))
                attn_chunk(0, PP1, 3)
                filler.append(gen_qkv(3))
                filler.append(gen_qkv(4))
                attn_chunk(1, PP1, 3)
                filler.append(gen_qkv(5))
                filler.append(gen_qkv(6))
                deferred_g = []
                attn_chunk(2, PP1, 3, deferred=deferred_g)
                cover = psB.tile([128, 512], F32, tag="B", name="coverps")
                for _ in range(10):
                    nc.tensor.matmul(
                        cover[:, 0:256], lhsT=warm_sb[:, 0:128], rhs=warm_sb,
                        start=True, stop=True,
                    )

            # ============ block 2: attention chunk 3 + projection ============
            with (
                tc.tile_pool(name="wpp", bufs=1) as WPP,
                tc.tile_pool(name="yf", bufs=8) as YF,
                tc.tile_pool(name="osb", bufs=2) as OP,
            ):
                for g_ in deferred_g:
                    g_()

                yf_tiles = {}

                def load_yf(tch):
                    for lh in range(HL):
                        yf_tiles[(tch, lh)] = YF.tile(
                            [128, NCORES, 512], BF16, tag="yf", name="yf_t"
                        )
                        nc.sync.dma_start(yf_tiles[(tch, lh)], yout_rs[lh][tch])

                load_yf(0)
                wp_sb = WPP.tile([128, 32, 512], BF16, tag="wp")
                for hf in range(2):
                    nc.scalar.dma_start(
                        wp_sb[:, :, hf * 256 : hf * 256 + 256],
                        wp_r[:, :, hf * 256 : hf * 256 + 256],
                    )

                def gen_proj(tch):
                    if tch < 3:
                        load_yf(tch + 1)
                    for ot in range(4):
                        ps = psBig.tile([128, 512], F32, tag="big")
                        n_mm = 0
                        for lh in range(HL):
                            for c in range(NCORES):
                                jt = 4 * c + lh
                                nc.tensor.matmul(
                                    ps,
                                    lhsT=wp_sb[:, jt, ot * 128 : ot * 128 + 128],
                                    rhs=yf_tiles[(tch, lh)][:, c, :],
                                    start=(n_mm == 0), stop=(n_mm == 31),
                                )
                                n_mm += 1
                                if n_mm % 3 == 0:
                                    yield
                        ob = OP.tile([128, 512], F32, tag="ob")
                        nc.scalar.activation(
                            ob, ps,
                            mybir.ActivationFunctionType.Identity,
                            bias=bp_sb[:, ot : ot + 1],
                        )
                        nc.sync.dma_start(
                            outT.ap()[ot * 128 : ot * 128 + 128,
                                      tch * 512 : tch * 512 + 512],
                            ob,
                        )
                        yield

                filler.append(gen_proj(0))
                filler.append(gen_proj(1))
                filler.append(gen_proj(2))
                attn_chunk(3, PP1, 2)
                for _ in gen_proj(3):
                    pass

    nc.compile()
    return nc


_NC_CACHE = {}


def _get_program(repeat=1):
    if repeat not in _NC_CACHE:
        _NC_CACHE[repeat] = _build_program(repeat)
    return _NC_CACHE[repeat]


def _bf16(a):
    return np.ascontiguousarray(a).astype(ml_dtypes.bfloat16)


def _f32(a):
    return np.ascontiguousarray(a, dtype=np.float32)


def _make_in_maps(x, cos, sin, W_attn, b_attn, W_proj, b_proj):
    x = np.asarray(x, dtype=np.float32)
    cos = np.asarray(cos, dtype=np.float32)
    sin = np.asarray(sin, dtype=np.float32)
    W_attn = np.asarray(W_attn, dtype=np.float32)
    b_attn = np.asarray(b_attn, dtype=np.float32)
    W_proj = np.asarray(W_proj, dtype=np.float32)
    b_proj = np.asarray(b_proj, dtype=np.float32)

    xT_b = _bf16(x[0].T)
    cosT = _bf16(cos.T)
    sinT = sin.T
    sin_pm = _bf16(np.concatenate([-sinT[: NE // 2], sinT[NE // 2 :]], axis=0))

    ii = np.arange(128)[:, None]
    jj = np.arange(128)[None, :]
    tri = (jj >= ii).astype(np.float32)  # 1 where q >= k (valid)

    Wr = W_attn.reshape(H, 3, HS, C)
    br = b_attn.reshape(H, 3, HS)

    in_maps = []
    for c in range(NCORES):
        hs = list(range(HL * c, HL * (c + 1)))
        wqk = np.concatenate([Wr[h, j] for h in hs for j in (0, 1)], axis=0)
        wv = np.concatenate([Wr[h, 2] for h in hs], axis=0)
        bqk = np.stack([br[h, j] for h in hs for j in (0, 1)], axis=1)
        bv = np.concatenate([br[h, 2] for h in hs], axis=0)
        wp = W_proj[512 * c : 512 * (c + 1), :]
        bp = b_proj[512 * c : 512 * (c + 1)].reshape(4, 128).T
        in_maps.append(
            {
                "xT": xT_b,
                "w_qkT": _bf16(wqk.T),
                "w_vT": _bf16(wv.T),
                "w_pT": _bf16(wp.T),
                "b_qk": _f32(bqk),
                "b_v": _bf16(np.tile(bv[None, :], (128, 1))),
                "b_p": _f32(bp),
                "cosT": cosT,
                "sin_pm": sin_pm,
                "tri128": _bf16(tri),
                "ident128": _bf16(np.eye(128, dtype=np.float32)),
            }
        )
    return in_maps


def kernel(**inputs):
    in_maps = _make_in_maps(**inputs)
    nc = _get_program()
    res = run_bass_kernel_spmd(nc, in_maps, core_ids=list(range(NCORES)))
    shards = [np.asarray(res.results[c]["outT"]) for c in range(NCORES)]
    out = np.concatenate(shards, axis=0)
    return np.ascontiguousarray(out.T)[None].astype(np.float32)


# revision 4
# speedup vs baseline: 1.0185x; 1.0185x over previous
"""Trainium2 Bass kernel: CausalSelfAttention (B=1, T=2048, C=4096, H=32, HS=128, NE=32).

Tensor-parallel over heads: 4 heads/core on 8 cores. Single streamed x pass
with all QKV weights resident. Attention S-tiles are *woven* with QKV (and
later projection) matmuls so the exp on the Activation engine never throttles
the PE. Masking is a post-exp multiplicative triangular mask (off the
S->exp critical chain). Softmax denominator via 1-column matmuls + tiny PE
transposes + PE broadcast. Warmup matmuls ramp the PE p-state during the
initial weight DMA.
"""

import sys

sys.path.insert(0, "/opt/trn_rl_repo")

from collections import deque

import numpy as np
import ml_dtypes

import concourse.bass as bass
import concourse.bacc as bacc
import concourse.mybir as mybir
from concourse import tile
from concourse.bass_utils import run_bass_kernel_spmd

BF16 = mybir.dt.bfloat16
F32 = mybir.dt.float32

B, T, C = 1, 2048, 4096
H, HS, NE = 32, 128, 32
NCORES = 8
HL = H // NCORES
SCALE = 1.0 / float(np.sqrt(HS))

NQC = 4
ROT32 = list(range(16, 32)) + list(range(16))
XCHUNKS = [(0, 512)] + [(512 + 256 * i, 256) for i in range(6)]


def _build_program(repeat=1, collective=True):
    nc = bacc.Bacc(
        "TRN2",
        target_bir_lowering=False,
        debug=False,
        num_devices=NCORES if collective else 1,
    )

    xT = nc.dram_tensor("xT", [C, T], BF16, kind="ExternalInput")
    w_qkT = nc.dram_tensor("w_qkT", [C, 2 * HL * 128], BF16, kind="ExternalInput")
    w_vT = nc.dram_tensor("w_vT", [C, HL * 128], BF16, kind="ExternalInput")
    w_pT = nc.dram_tensor("w_pT", [C, 512], BF16, kind="ExternalInput")
    b_qk = nc.dram_tensor("b_qk", [128, 2 * HL], F32, kind="ExternalInput")
    b_v = nc.dram_tensor("b_v", [128, HL * 128], BF16, kind="ExternalInput")
    b_p = nc.dram_tensor("b_p", [128, 4], F32, kind="ExternalInput")
    cosT = nc.dram_tensor("cosT", [NE, T], BF16, kind="ExternalInput")
    sin_pm = nc.dram_tensor("sin_pm", [NE, T], BF16, kind="ExternalInput")
    tri128 = nc.dram_tensor("tri128", [128, 128], BF16, kind="ExternalInput")
    ident128 = nc.dram_tensor("ident128", [128, 128], BF16, kind="ExternalInput")
    outT = nc.dram_tensor("outT", [512, T], F32, kind="ExternalOutput")

    y_ins = [
        [nc.dram_tensor(f"y_in_{lh}_{qc}", [128, 512], BF16) for qc in range(NQC)]
        for lh in range(HL)
    ]
    y_outs = [
        [
            nc.dram_tensor(
                f"y_out_{lh}_{qc}", [NCORES * 128, 512], BF16, addr_space="Shared"
            )
            for qc in range(NQC)
        ]
        for lh in range(HL)
    ]

    xT_r = xT.ap().rearrange("(ct p) t -> p ct t", p=128)
    wqk_r = w_qkT.ap().rearrange("(ct p) r -> p ct r", p=128)
    wv_r = w_vT.ap().rearrange("(ct p) r -> p ct r", p=128)
    wp_r = w_pT.ap().rearrange("(ct p) r -> p ct r", p=128)
    yout_rs = [
        [y.ap().rearrange("(c p) t -> p c t", p=128) for y in row] for row in y_outs
    ]

    with tile.TileContext(nc) as tc:
      for _rep in range(repeat):
        with (
            tc.tile_pool(name="persist", bufs=1) as P0,
            tc.tile_pool(name="qtp", bufs=2) as QTP,
            tc.tile_pool(name="ytile", bufs=5) as YP,
            tc.tile_pool(name="stat", bufs=1) as ST,
            tc.tile_pool(name="pp1", bufs=24) as PP1,
            tc.tile_pool(name="psQK", bufs=1, space="PSUM") as psQK,
            tc.tile_pool(name="psS", bufs=2, space="PSUM") as psS,
            tc.tile_pool(name="psBig", bufs=2, space="PSUM") as psBig,
            tc.tile_pool(name="psDT", bufs=1, space="PSUM") as psDT,
            tc.tile_pool(name="psRow", bufs=1, space="PSUM") as psRow,
            tc.tile_pool(name="psB", bufs=1, space="PSUM") as psB,
        ):
            kT_sb = P0.tile([128, HL, T], BF16, tag="kT")
            v_sb = P0.tile([128, T // 128, 512], BF16, tag="v")
            bp_sb = P0.tile([128, 4], F32, tag="bp")
            tri_sb = P0.tile([128, 128], BF16, tag="tri")
            id_sb = P0.tile([128, 128], BF16, tag="id")
            ones_c = P0.tile([128, 1], BF16, tag="onc")
            ones_r = P0.tile([1, 128], BF16, tag="onr")
            warm_sb = P0.tile([128, 256], BF16, tag="warm")

            qT = {}

            # ---- weave machinery: filler generators yield after PE quanta ----
            filler = deque()

            def pull(n=1):
                for _ in range(n):
                    while filler:
                        try:
                            next(filler[0])
                            return
                        except StopIteration:
                            filler.popleft()
                    return

            def drain_filler():
                while filler:
                    try:
                        next(filler[0])
                    except StopIteration:
                        filler.popleft()

            def attn_chunk(qc, PP, pull_n, deferred=None, p_init=None,
                           only_S0=False):
                nkt = 4 * qc + 4
                p_tiles = {} if p_init is None else p_init

                def S_head(lh, kts):
                    for kt in kts:
                        r = kt - 4 * qc
                        c0 = 128 * r if r >= 1 else 0
                        ps = psS.tile([128, 512], F32, tag="S")
                        nc.tensor.matmul(
                            ps[:, c0:512],
                            lhsT=kT_sb[:, lh, kt * 128 : kt * 128 + 128],
                            rhs=qT[qc][:, lh, c0:512],
                            start=True, stop=True,
                        )
                        pt = PP.tile([128, 512], BF16, tag="P")
                        nc.scalar.activation(
                            pt[:, c0:512], ps[:, c0:512],
                            mybir.ActivationFunctionType.Exp,
                            scale=SCALE,
                        )
                        if r >= 0:
                            # multiplicative causal mask, off the exp chain
                            nc.vector.tensor_tensor(
                                pt[:, 128 * r : 128 * r + 128],
                                pt[:, 128 * r : 128 * r + 128],
                                tri_sb, op=mybir.AluOpType.mult,
                            )
                            if qc == 0 and r >= 1:
                                nc.gpsimd.memset(pt[:, 0 : 128 * r], 0.0)
                        p_tiles[(lh, kt)] = (pt, c0)
                        pull(pull_n)

                def DY_head(lh):
                    # column-major: one sequential accumulation group per
                    # psdt column (interleaved groups in one bank are not
                    # supported by the hardware accumulation tracker)
                    psdt = psDT.tile([128, 4], F32, tag="DT")
                    for s in range(4):
                        kts = [
                            kt for kt in range(nkt)
                            if p_tiles[(lh, kt)][1] // 128 <= s
                        ]
                        for i, kt in enumerate(kts):
                            pt, _ = p_tiles[(lh, kt)]
                            nc.tensor.matmul(
                                psdt[:, s : s + 1],
                                lhsT=pt[:, 128 * s : 128 * s + 128],
                                rhs=ones_c,
                                start=(i == 0), stop=(i == len(kts) - 1),
                            )
                    rcp32 = ST.tile([128, 4], F32, tag="rcp32")
                    nc.vector.reciprocal(rcp32, psdt)
                    rcpb = ST.tile([128, 4], BF16, tag="rcpb")
                    nc.vector.tensor_copy(rcpb, rcp32)
                    if qc == 0:
                        y_order = [
                            (kt, 0, kt == 0, kt == nkt - 1) for kt in range(nkt)
                        ]
                    else:
                        y_order = [(4 * qc, 0, True, False)]
                        for r in range(1, 4):
                            y_order.append((4 * qc + r, 128 * r, False, False))
                        for kt in range(4 * qc):
                            y_order.append((kt, 0, False, kt == 4 * qc - 1))
                    ys = psBig.tile([128, 512], F32, tag="big")
                    for kt, c0, st_, sp_ in y_order:
                        pt, _ = p_tiles[(lh, kt)]
                        nc.tensor.matmul(
                            ys[:, c0:512],
                            lhsT=v_sb[:, kt, lh * 128 : lh * 128 + 128],
                            rhs=pt[:, c0:512],
                            start=st_, stop=sp_,
                        )
                    psrow = psRow.tile([1, 512], BF16, tag="row")
                    for s in range(4):
                        nc.tensor.transpose(
                            psrow[:, 128 * s : 128 * s + 128],
                            rcpb[:, s : s + 1],
                            id_sb,
                        )
                    row_sb = ST.tile([1, 512], BF16, tag="row_sb")
                    nc.vector.tensor_copy(row_sb, psrow)
                    psb = psB.tile([128, 512], F32, tag="B")
                    nc.tensor.matmul(
                        psb, lhsT=ones_r, rhs=row_sb, start=True, stop=True
                    )
                    bcs = ST.tile([128, 512], BF16, tag="bcs")
                    nc.scalar.copy(bcs, psb)
                    yt = YP.tile([128, 512], BF16, tag="yt")
                    nc.vector.tensor_tensor(yt, ys, bcs, op=mybir.AluOpType.mult)

                    def gather(lh=lh):
                        nc.sync.dma_start(y_ins[lh][qc].ap(), yt)
                        if collective:
                            nc.gpsimd.collective_compute(
                                "AllGather",
                                mybir.AluOpType.bypass,
                                replica_groups=[list(range(NCORES))],
                                ins=[y_ins[lh][qc].ap().opt()],
                                outs=[y_outs[lh][qc].ap().opt()],
                            )
                        else:
                            nc.sync.dma_start(
                                y_outs[lh][qc].ap()[0:128, :], y_ins[lh][qc].ap()
                            )

                    if deferred is None:
                        gather()
                    else:
                        deferred.append(gather)

                half = max(nkt // 2, 1)
                if only_S0:
                    S_head(0, range(nkt))
                    return p_tiles
                if p_init is None:
                    S_head(0, range(nkt))
                S_head(1, range(nkt))
                DY_head(0)
                S_head(2, range(nkt))
                DY_head(1)
                S_head(3, range(half))        # cap P-pool liveness
                if qc < 3:
                    drain_filler()
                DY_head(2)
                S_head(3, range(half, nkt))
                DY_head(3)
                drain_filler()

            # ============ block 1: QKV + attention chunks 0-2 ============
            with (
                tc.tile_pool(name="wpool", bufs=1) as WP,
                tc.tile_pool(name="xp", bufs=2) as XP,
                tc.tile_pool(name="rope", bufs=2) as RP,
            ):
                wqk_sb = WP.tile([128, 32, 1024], BF16, tag="wqk")
                wv_sb = WP.tile([128, 32, 512], BF16, tag="wv")
                bqk_sb = WP.tile([128, 2 * HL], F32, tag="bqk")
                bv_sb = WP.tile([128, HL * 128], BF16, tag="bv")

                def load_cs(t0):
                    cc = RP.tile([NE, 256], BF16, tag="ccos", name="cc_t")
                    ss = RP.tile([NE, 256], BF16, tag="csin", name="ss_t")
                    nc.sync.dma_start(cc, cosT.ap()[:, t0 : t0 + 256])
                    nc.sync.dma_start(ss, sin_pm.ap()[:, t0 : t0 + 256])
                    return cc, ss

                nc.vector.memset(warm_sb, 0.0)
                nc.vector.memset(ones_c, 1.0)
                nc.vector.memset(ones_r, 1.0)

                def rope_inplace(dest32, cc, ss):
                    n = dest32.shape[-1]
                    rot = RP.tile([NE, n], BF16, tag="rot")
                    nc.vector.stream_shuffle(rot, dest32, mask=ROT32)
                    tcos = RP.tile([NE, n], BF16, tag="tcos")
                    nc.vector.tensor_tensor(
                        tcos, dest32, cc, op=mybir.AluOpType.mult
                    )
                    tsin = RP.tile([NE, n], BF16, tag="tsin")
                    nc.vector.tensor_tensor(
                        tsin, rot, ss, op=mybir.AluOpType.mult
                    )
                    nc.vector.tensor_tensor(
                        dest32, tcos, tsin, op=mybir.AluOpType.add
                    )

                def qk_drain(ps, rt, ts, t0, tn, cs_tiles):
                    lh, is_k = rt // 2, rt % 2
                    if is_k:
                        dsl = kT_sb[:, lh, ts]
                    else:
                        q0 = t0 % 512
                        dsl = qT[t0 // 512][:, lh, q0 : q0 + tn]
                    nc.scalar.activation(
                        dsl, ps,
                        mybir.ActivationFunctionType.Identity,
                        bias=bqk_sb[:, rt : rt + 1],
                    )
                    for hi, h0 in enumerate(range(0, tn, 256)):
                        cc, ss = cs_tiles[hi]
                        rope_inplace(dsl[0:NE, h0 : h0 + 256], cc, ss)

                # ---- chunk 0 (512 tokens, paced against the weight DMA) ----
                def run_chunk0():
                    wps = psB.tile([128, 512], F32, tag="B", name="warmps")
                    for _ in range(26):
                        nc.tensor.matmul(
                            wps[:, 0:256], lhsT=warm_sb[:, 0:128], rhs=warm_sb,
                            start=True, stop=True,
                        )
                    xt0a = XP.tile([128, 32, 256], BF16, tag="xt", name="xt0a")
                    xt0b = XP.tile([128, 32, 256], BF16, tag="xt", name="xt0b")
                    for g in range(8):
                        gs = slice(4 * g, 4 * g + 4)
                        nc.sync.dma_start(wqk_sb[:, gs, :], wqk_r[:, gs, :])
                        nc.scalar.dma_start(xt0a[:, gs, :], xT_r[:, gs, 0:256])
                        nc.scalar.dma_start(xt0b[:, gs, :], xT_r[:, gs, 256:512])
                        if g == 0:
                            # constants needed by the first qk drains
                            nc.sync.dma_start(bqk_sb, b_qk.ap())
                            cs0 = [load_cs(0), load_cs(256)]
                    for g in range(4):
                        gs = slice(8 * g, 8 * g + 8)
                        nc.sync.dma_start(wv_sb[:, gs, :], wv_r[:, gs, :])
                        if g == 0:
                            nc.sync.dma_start(bv_sb, b_v.ap())
                    nc.sync.dma_start(tri_sb, tri128.ap())
                    nc.sync.dma_start(id_sb, ident128.ap())
                    nc.sync.dma_start(bp_sb, b_p.ap())

                    qT[0] = QTP.tile([128, HL, 512], BF16, tag="qT", name="qt_c")
                    for rts, pools in (
                        (range(0, 6),
                         ((psS, "S"), (psS, "S"), (psBig, "big"),
                          (psBig, "big"), (psB, "B"), (psQK, "qk"))),
                        (range(6, 8), ((psS, "S"), (psS, "S"))),
                    ):
                        tiles = {}
                        for rt, (pool, ptag) in zip(rts, pools):
                            tiles[rt] = pool.tile(
                                [128, 512], F32, tag=ptag, name=f"c0ps{rt}",
                            )
                        # one sequential accumulation group per column half
                        for hb, xs in ((0, xt0a), (1, xt0b)):
                            for g in range(8):
                                for rt in rts:
                                    for ct in range(4 * g, 4 * g + 4):
                                        nc.tensor.matmul(
                                            tiles[rt][:, 256 * hb : 256 * hb + 256],
                                            lhsT=wqk_sb[:, ct,
                                                        rt * 128 : rt * 128 + 128],
                                            rhs=xs[:, ct, :],
                                            start=(ct == 0), stop=(ct == 31),
                                        )
                        for rt in rts:
                            qk_drain(tiles[rt], rt, slice(0, 512), 0, 512, cs0)
                    for st in range(4):
                        ps = psBig.tile([128, 512], F32, tag="big", name=f"c0v{st}")
                        xs = (xt0a, xt0b)[st // 2]
                        x0 = (st % 2) * 128
                        for ct in range(32):
                            nc.tensor.matmul(
                                ps,
                                lhsT=xs[:, ct, x0 : x0 + 128],
                                rhs=wv_sb[:, ct, :],
                                start=(ct == 0), stop=(ct == 31),
                            )
                        nc.vector.tensor_tensor(
                            v_sb[:, st, :], ps, bv_sb, op=mybir.AluOpType.add
                        )

                run_chunk0()

                def gen_qkv(xc):
                    t0, tn = XCHUNKS[xc]
                    ts = slice(t0, t0 + tn)
                    if t0 % 512 == 0:
                        qT[t0 // 512] = QTP.tile(
                            [128, HL, 512], BF16, tag="qT", name="qt_c"
                        )
                    xt = XP.tile([128, 32, 256], BF16, tag="xt")
                    nc.scalar.dma_start(xt, xT_r[:, :, ts])
                    cs_t = [load_cs(t0)]
                    for rt in range(8):
                        ps = psQK.tile([128, 256], F32, tag="qk")
                        for ct in range(32):
                            nc.tensor.matmul(
                                ps,
                                lhsT=wqk_sb[:, ct, rt * 128 : rt * 128 + 128],
                                rhs=xt[:, ct, :],
                                start=(ct == 0), stop=(ct == 31),
                            )
                            if ct % 6 == 5:
                                yield
                        qk_drain(ps, rt, ts, t0, tn, cs_t)
                        yield
                    for sti in range(tn // 128):
                        st = t0 // 128 + sti
                        ps = psBig.tile([128, 512], F32, tag="big")
                        for ct in range(32):
                            nc.tensor.matmul(
                                ps,
                                lhsT=xt[:, ct, sti * 128 : sti * 128 + 128],
                                rhs=wv_sb[:, ct, :],
                                start=(ct == 0), stop=(ct == 31),
                            )
                            if ct % 3 == 2:
                                yield
                        nc.vector.tensor_tensor(
                            v_sb[:, st, :], ps, bv_sb, op=mybir.AluOpType.add
                        )
                        yield

                filler.append(gen_qkv(1))
                filler.append(gen_qkv(2))
                attn_chunk(0, PP1, 3)
                filler.append(gen_qkv(3))
                filler.append(gen_qkv(4))
                attn_chunk(1, PP1, 3)
                filler.append(gen_qkv(5))
                filler.append(gen_qkv(6))
                deferred_g = []
                attn_chunk(2, PP1, 3, deferred=deferred_g)
                p3_init = attn_chunk(3, PP1, 0, only_S0=True)
                cover = psB.tile([128, 512], F32, tag="B", name="coverps")
                for _ in range(10):
                    nc.tensor.matmul(
                        cover[:, 0:256], lhsT=warm_sb[:, 0:128], rhs=warm_sb,
                        start=True, stop=True,
                    )

            # ============ block 2: attention chunk 3 + projection ============
            with (
                tc.tile_pool(name="wpp", bufs=1) as WPP,
                tc.tile_pool(name="yf", bufs=8) as YF,
                tc.tile_pool(name="osb", bufs=2) as OP,
            ):
                def gen_gathers():
                    for g_ in deferred_g:
                        g_()
                    yield

                yf_tiles = {}

                def load_yf(tch):
                    for lh in range(HL):
                        yf_tiles[(tch, lh)] = YF.tile(
                            [128, NCORES, 512], BF16, tag="yf", name="yf_t"
                        )
                        nc.sync.dma_start(yf_tiles[(tch, lh)], yout_rs[lh][tch])

                load_yf(0)
                wp_sb = WPP.tile([128, 32, 512], BF16, tag="wp")
                for hf in range(2):
                    nc.scalar.dma_start(
                        wp_sb[:, :, hf * 256 : hf * 256 + 256],
                        wp_r[:, :, hf * 256 : hf * 256 + 256],
                    )

                def gen_proj(tch):
                    if tch < 3:
                        load_yf(tch + 1)
                    for ot in range(4):
                        ps = psBig.tile([128, 512], F32, tag="big")
                        n_mm = 0
                        for lh in range(HL):
                            for c in range(NCORES):
                                jt = 4 * c + lh
                                nc.tensor.matmul(
                                    ps,
                                    lhsT=wp_sb[:, jt, ot * 128 : ot * 128 + 128],
                                    rhs=yf_tiles[(tch, lh)][:, c, :],
                                    start=(n_mm == 0), stop=(n_mm == 31),
                                )
                                n_mm += 1
                                if n_mm % 3 == 0:
                                    yield
                        ob = OP.tile([128, 512], F32, tag="ob")
                        nc.scalar.activation(
                            ob, ps,
                            mybir.ActivationFunctionType.Identity,
                            bias=bp_sb[:, ot : ot + 1],
                        )
                        nc.sync.dma_start(
                            outT.ap()[ot * 128 : ot * 128 + 128,
                                      tch * 512 : tch * 512 + 512],
                            ob,
                        )
                        yield

                filler.append(gen_gathers())
                filler.append(gen_proj(0))
                filler.append(gen_proj(1))
                filler.append(gen_proj(2))
                attn_chunk(3, PP1, 2, p_init=p3_init)
                for _ in gen_proj(3):
                    pass

    nc.compile()
    return nc


_NC_CACHE = {}


def _get_program(repeat=1):
    if repeat not in _NC_CACHE:
        _NC_CACHE[repeat] = _build_program(repeat)
    return _NC_CACHE[repeat]


def _bf16(a):
    return np.ascontiguousarray(a).astype(ml_dtypes.bfloat16)


def _f32(a):
    return np.ascontiguousarray(a, dtype=np.float32)


def _make_in_maps(x, cos, sin, W_attn, b_attn, W_proj, b_proj):
    x = np.asarray(x, dtype=np.float32)
    cos = np.asarray(cos, dtype=np.float32)
    sin = np.asarray(sin, dtype=np.float32)
    W_attn = np.asarray(W_attn, dtype=np.float32)
    b_attn = np.asarray(b_attn, dtype=np.float32)
    W_proj = np.asarray(W_proj, dtype=np.float32)
    b_proj = np.asarray(b_proj, dtype=np.float32)

    xT_b = _bf16(x[0].T)
    cosT = _bf16(cos.T)
    sinT = sin.T
    sin_pm = _bf16(np.concatenate([-sinT[: NE // 2], sinT[NE // 2 :]], axis=0))

    ii = np.arange(128)[:, None]
    jj = np.arange(128)[None, :]
    tri = (jj >= ii).astype(np.float32)  # 1 where q >= k (valid)

    Wr = W_attn.reshape(H, 3, HS, C)
    br = b_attn.reshape(H, 3, HS)

    in_maps = []
    for c in range(NCORES):
        hs = list(range(HL * c, HL * (c + 1)))
        wqk = np.concatenate([Wr[h, j] for h in hs for j in (0, 1)], axis=0)
        wv = np.concatenate([Wr[h, 2] for h in hs], axis=0)
        bqk = np.stack([br[h, j] for h in hs for j in (0, 1)], axis=1)
        bv = np.concatenate([br[h, 2] for h in hs], axis=0)
        wp = W_proj[512 * c : 512 * (c + 1), :]
        bp = b_proj[512 * c : 512 * (c + 1)].reshape(4, 128).T
        in_maps.append(
            {
                "xT": xT_b,
                "w_qkT": _bf16(wqk.T),
                "w_vT": _bf16(wv.T),
                "w_pT": _bf16(wp.T),
                "b_qk": _f32(bqk),
                "b_v": _bf16(np.tile(bv[None, :], (128, 1))),
                "b_p": _f32(bp),
                "cosT": cosT,
                "sin_pm": sin_pm,
                "tri128": _bf16(tri),
                "ident128": _bf16(np.eye(128, dtype=np.float32)),
            }
        )
    return in_maps


def kernel(**inputs):
    in_maps = _make_in_maps(**inputs)
    nc = _get_program()
    res = run_bass_kernel_spmd(nc, in_maps, core_ids=list(range(NCORES)))
    shards = [np.asarray(res.results[c]["outT"]) for c in range(NCORES)]
    out = np.concatenate(shards, axis=0)
    return np.ascontiguousarray(out.T)[None].astype(np.float32)
